# revision 6
# baseline (speedup 1.0000x reference)
"""Trainium2 Bass kernel for nn_BoundaryKDV7 (boundary KL-divergence loss).

Contract: kernel(**inputs) takes the FULL inputs
    preds_S [8, 14, 512, 512] f32
    preds_T [8, 14, 512, 512] f32
    gt_labels [8, 1, 512, 512] i32
and returns the scalar f32 loss. Batch dim is sharded across 8
NeuronCores (pure data parallel); each core emits per-column partial
sums of w*kl which the host reduces to the scalar.

Math (matches the reference up to fp reassociation):
  kl_pix = W/ZT + ln(ZS/ZT), with
    ZT = sum_c exp(t_c), ZS = sum_c exp(s_c), W = sum_c exp(t_c)(t_c - s_c)
  loss = sum_p w_p * kl_p, where the per-pixel weight
    w_p = valid_{gt_p} / (C * max(n_{gt_p}, 1)) if p is a boundary pixel
  folds the per-class boundary masks, counts and validity — all derivable
  from gt_labels alone — into one plane computed on the host (gt is 3%
  of the input bytes; the preds math stays on device).

Numerics (validated host-side vs the f64 reference: rel err ~2e-5
against a 2e-2 gate):
  - Inputs are downconverted to f16 on the host (halves HBM traffic;
    inputs are N(0,1) so rounding noise washes out over ~240k boundary
    pixels).
  - exp(s) is only needed for the ZS channel-sum, so it tolerates a
    fast approximation: a Schraudolph-style bit-trick on the DVE
    (i16 = round(s*1024/ln2 + MU), bitcast to f16), with MU chosen so
    the relative error (+-2% sawtooth) is zero-mean over the uniform
    mantissa-phase distribution. This moves half the exp work off the
    ACT engine (the v2a bottleneck) onto a 4x-mode DVE tensor_scalar.
  - exp(t) stays exact on ACT (it also feeds the W stream).

On-device layout (per core, P = 262144 pixels):
  Channel-on-partition tiles [112, 2048]: partition = (pixgroup j in
  0..7, channel c in 0..13), each pixgroup row holds 2048 consecutive
  pixels. The 14-channel sums (ZT, ZS, W) are computed on the
  TensorEngine with a constant 0/1 selector lhsT so the PSUM output
  lands pixel-major: [128, 512] per 65536-pixel superchunk, psum row
  r', col f <-> pixel 65536*s + 512*r' + f.
  d = t - s runs on GPSIMD for 12 of 16 quads to balance DVE/Pool.
  Finals per superchunk: lnZT/lnZS/r=1/ZT on ACT, then on DVE
  g = lnZS - lnZT, h = W*r, kl = h + g, wkl = w*kl; one ones-column
  matmul accumulates per-column sums into a [1, 512] PSUM row; the
  host sums the 512 columns.
"""

import numpy as np
from contextlib import ExitStack

B, C, H, W = 8, 14, 512, 512
P = H * W              # 262144 pixels per sample
FQ = 2048              # free dim of a quad-tile
NSC = 4                # superchunks (65536 px each)
N_CORES = 8
WSCALE = 16384.0       # host weight scale: keeps w ~0.06 in f16

# Schraudolph f16 exp: i16 = round(x*SIG + MU); bitcast to f16.
# MU includes the zero-mean correction log2(E[(1+phi) 2^-phi]) so the
# +-2% piecewise-linear sawtooth has no systematic bias.
SIG = 1024.0 / float(np.log(2.0))
MU = 15301.0864

_CACHE = {}


def _build_sel() -> np.ndarray:
    """Phase-B selector weights [112, 16*128] f16.

    Partition layout is channel-major: p = ch*8 + j (so the input DMA
    iterates channels outermost and reads 8 contiguous pixgroup rows per
    channel). Block i' = 4*(Q%4) + c is the lhsT for matmul (quad Q,
    512-chunk c): sel[(ch*8+j), i', m] = 1 iff m == 32*(Q%4) + 4*j + c,
    mapping pixel 16384*Q + 2048*j + 512*c + f to psum row
    (32*(Q%4)+4*j+c), col f.
    """
    sel = np.zeros((112, 16, 128), np.float16)
    for qm in range(4):
        for c in range(4):
            blk = 4 * qm + c
            for j in range(8):
                row = 32 * qm + 4 * j + c
                sel[j::8, blk, row] = 1.0
    return np.ascontiguousarray(sel.reshape(112, 16 * 128))


def _patched_act_tables(orig_fn):
    """Force Exp and Ln to resolve to the one table set containing both
    (natural_log_exp_and_others) so the kernel never switches sets."""
    def wrapper(arch):
        import concourse.mybir as mybir
        tabs = orig_fn(arch)
        both = "natural_log_exp_and_others"
        if both in tabs:
            for name, funcs in tabs.items():
                if name != both:
                    funcs.discard(mybir.ActivationFunctionType.Exp)
                    funcs.discard(mybir.ActivationFunctionType.Ln)
        return tabs
    return wrapper


def _emit(nc, tc, S, T, WD, SEL, OUT):
    import concourse.bass as bass
    from concourse import mybir

    f32 = mybir.dt.float32
    f16 = mybir.dt.float16
    i16 = mybir.dt.int16
    Act = mybir.ActivationFunctionType
    Alu = mybir.AluOpType

    with ExitStack() as ctx:
        consts = ctx.enter_context(tc.tile_pool(name="consts", bufs=1))
        planes = ctx.enter_context(tc.tile_pool(name="planes", bufs=1))
        inpool = ctx.enter_context(tc.tile_pool(name="inpool", bufs=4))
        midpool = ctx.enter_context(tc.tile_pool(name="midpool", bufs=3))
        finpool = ctx.enter_context(tc.tile_pool(name="finpool", bufs=3))
        psum = ctx.enter_context(
            tc.tile_pool(name="psum", bufs=2, space=bass.MemorySpace.PSUM))
        psumc = ctx.enter_context(
            tc.tile_pool(name="psumc", bufs=1, space=bass.MemorySpace.PSUM))

        # ---- constants ----
        sel_sb = consts.tile([112, 16 * 128], f16)
        nc.sync.dma_start(sel_sb[:], SEL[:])
        ones_sb = consts.tile([128, 1], f16)
        nc.vector.memset(ones_sb[:], 1.0)
        # per-pixel weights, pixel-major per superchunk: [r, s, f]
        wt = consts.tile([128, NSC, 512], f16)
        nc.sync.dma_start(wt[:], WD[:].rearrange("r (s f) -> r s f", s=NSC))

        # ---- final reduction target: [1, 512] PSUM row ----
        acc = psumc.tile([1, 512], f32)

        Sr = S.rearrange("c (q j f) -> q c j f", j=8, f=FQ)
        Tr = T.rearrange("c (q j f) -> q c j f", j=8, f=FQ)

        def emit_quad(s, qq, psZT, psZS, psW):
            Q = 4 * s + qq
            St = inpool.tile([112, FQ], f16, tag="St")
            Tt = inpool.tile([112, FQ], f16, tag="Tt")
            nc.sync.dma_start(St[:], Sr[Q])
            nc.sync.dma_start(Tt[:], Tr[Q])
            d = midpool.tile([112, FQ], f16, tag="d")
            # 12 of 16 quads compute d on GPSIMD to unload the DVE
            deng = nc.vector if qq == 0 else nc.gpsimd
            deng.tensor_tensor(d[:], Tt[:], St[:], Alu.subtract)
            eT = midpool.tile([112, FQ], f16, tag="eT")
            nc.scalar.activation(eT[:], Tt[:], Act.Exp)
            eS = midpool.tile([112, FQ], i16, tag="eS")
            nc.vector.tensor_scalar(eS[:], St[:], SIG, MU, Alu.mult, Alu.add)
            eSf = eS[:].bitcast(f16)
            m = midpool.tile([112, FQ], f16, tag="m")
            nc.vector.tensor_mul(m[:], eT[:], d[:])
            for cc in range(4):
                blk = 4 * qq + cc
                selap = sel_sb[:, blk * 128:(blk + 1) * 128]
                st = (qq == 0 and cc == 0)
                sp = (qq == 3 and cc == 3)
                cs = slice(cc * 512, (cc + 1) * 512)
                nc.tensor.matmul(psZT[:], selap, eT[:, cs],
                                 start=st, stop=sp)
                nc.tensor.matmul(psZS[:], selap, eSf[:, cs],
                                 start=st, stop=sp)
                nc.tensor.matmul(psW[:], selap, m[:, cs],
                                 start=st, stop=sp)

        def make_finals_parts(s, psZT, psZS, psW):
            """Finals of superchunk s as closures, interleaved between the
            next superchunk's quads for a smoother static schedule."""
            st = {}

            def part0():
                lnZT = finpool.tile([128, 512], f32, tag="lnZT")
                lnZS = finpool.tile([128, 512], f32, tag="lnZS")
                r = finpool.tile([128, 512], f32, tag="r")
                nc.scalar.activation(lnZT[:], psZT[:], Act.Ln)
                nc.scalar.activation(lnZS[:], psZS[:], Act.Ln)
                nc.scalar.activation(r[:], lnZT[:], Act.Exp, scale=-1.0)
                st["lnZT"], st["lnZS"], st["r"] = lnZT, lnZS, r

            def part1():
                g = finpool.tile([128, 512], f16, tag="g")
                h = finpool.tile([128, 512], f16, tag="h")
                nc.vector.tensor_sub(g[:], st["lnZS"][:], st["lnZT"][:])
                nc.vector.tensor_mul(h[:], psW[:], st["r"][:])
                kl = finpool.tile([128, 512], f16, tag="kl")
                wkl = finpool.tile([128, 512], f16, tag="wkl")
                nc.vector.tensor_add(kl[:], h[:], g[:])
                nc.vector.tensor_mul(wkl[:], kl[:], wt[:, s, :])
                nc.tensor.matmul(acc[:], ones_sb[:], wkl[:],
                                 start=(s == 0), stop=(s == NSC - 1))

            return [part0, part1]

        # software pipeline: superchunk s's finals are emitted between
        # superchunk s+1's quads so no engine stalls on the
        # PE -> DVE -> ACT -> DVE -> PE round-trip at superchunk edges
        pending = None
        for s in range(NSC):
            psZT = psum.tile([128, 512], f32, tag="psZT")
            psZS = psum.tile([128, 512], f32, tag="psZS")
            psW = psum.tile([128, 512], f32, tag="psW")
            for qq in range(4):
                emit_quad(s, qq, psZT, psZS, psW)
                if pending is not None and qq < 2:
                    pending[qq]()
            pending = make_finals_parts(s, psZT, psZS, psW)
        for part in pending:
            part()

        acc_sb = planes.tile([1, 512], f32)
        nc.vector.tensor_copy(acc_sb[:], acc[:])
        nc.sync.dma_start(OUT[:], acc_sb[:])


def _build_nc():
    import concourse.bacc as bacc
    import concourse.tile as tile
    import concourse.hw_specs as hw_specs
    from concourse import mybir

    if not getattr(bacc, "_act_tables_patched", False):
        bacc.get_activation_tables = _patched_act_tables(
            hw_specs.get_activation_tables)
        bacc._act_tables_patched = True

    f32 = mybir.dt.float32
    f16 = mybir.dt.float16

    nc = bacc.Bacc("TRN2", target_bir_lowering=False, debug=False)
    S = nc.declare_dram_parameter("preds_s", [C, P], f16, isOutput=False)
    T = nc.declare_dram_parameter("preds_t", [C, P], f16, isOutput=False)
    WD = nc.declare_dram_parameter("wpix", [128, NSC * 512], f16,
                                   isOutput=False)
    SEL = nc.declare_dram_parameter("sel", [112, 16 * 128], f16,
                                    isOutput=False)
    OUT = nc.declare_dram_parameter("partials", [1, 512], f32, isOutput=True)
    with tile.TileContext(nc) as tc:
        _emit(nc, tc, S, T, WD, SEL, OUT)
    nc.compile()
    return nc


def _get_nc():
    if "nc" not in _CACHE:
        _CACHE["nc"] = _build_nc()
    return _CACHE["nc"]


def _weights(gt: np.ndarray) -> np.ndarray:
    """Per-pixel loss weights [128, NSC*512] f16 for one sample's gt.

    w_p = WSCALE * valid_k / (C * max(n_k, 1)) for boundary pixels of
    class k = gt_p (masks are disjoint across k), 0 elsewhere.
    Matches the reference: boundary = mask ^ erosion (cross, zero
    border); valid_k = (sum of flat boundary indices) > 0.
    Layout is pixel-major per superchunk: out[r, s*512+f] = w[128*s+r, f].
    """
    w = np.zeros((H, W), np.float32)
    mp = np.zeros((H + 2, W + 2), bool)
    for k in range(1, C):
        m = gt == k
        mp[1:-1, 1:-1] = m
        eroded = (m & mp[:-2, 1:-1] & mp[2:, 1:-1]
                  & mp[1:-1, :-2] & mp[1:-1, 2:])
        boundary = m ^ eroded
        n = int(boundary.sum())
        if n == 0:
            continue
        # valid iff any boundary pixel has flat index > 0
        if n == 1 and boundary[0, 0]:
            continue
        w[boundary] = WSCALE / (C * n)
    return np.ascontiguousarray(
        w.reshape(NSC, 128, 512).transpose(1, 0, 2).reshape(128, NSC * 512)
    ).astype(np.float16)


def make_in_maps(preds_S, preds_T, gt_labels):
    """Shard the full inputs into per-core input maps (host-side layout)."""
    gt = np.asarray(gt_labels)[:, 0]                       # [nb, 512, 512]
    nb = gt.shape[0]
    sel = _build_sel()
    pS = np.asarray(preds_S, np.float32).reshape(nb, C, P).astype(np.float16)
    pT = np.asarray(preds_T, np.float32).reshape(nb, C, P).astype(np.float16)
    return [
        {"preds_s": np.ascontiguousarray(pS[b]),
         "preds_t": np.ascontiguousarray(pT[b]),
         "wpix": _weights(gt[b]), "sel": sel}
        for b in range(nb)
    ]


def postprocess(partials_per_core) -> np.float32:
    """Reduce per-core [1, 512] partials to the scalar loss."""
    loss = 0.0
    for part in partials_per_core:
        loss += float(part.astype(np.float64).sum()) / WSCALE
    return np.float32(loss)


def _run(inputs, trace=False, trace_kwargs=None):
    from concourse.bass_utils import run_bass_kernel_spmd

    nc = _get_nc()
    in_maps = make_in_maps(inputs["preds_S"], inputs["preds_T"],
                           inputs["gt_labels"])
    res = run_bass_kernel_spmd(nc, in_maps, list(range(len(in_maps))),
                               trace=trace, **(trace_kwargs or {}))
    parts = [res.results[b]["partials"] for b in range(len(in_maps))]
    loss = postprocess(parts)
    return loss, res


def kernel(preds_S, preds_T, gt_labels):
    assert preds_S.shape == (B, C, H, W), preds_S.shape
    loss, _ = _run({"preds_S": preds_S, "preds_T": preds_T,
                    "gt_labels": gt_labels})
    return loss


# revision 13
# speedup vs baseline: 1.1824x; 1.1824x over previous
"""Trainium2 Bass kernel for nn_BoundaryKDV7 (boundary KL-divergence loss).

Contract: kernel(**inputs) takes the FULL inputs
    preds_S [8, 14, 512, 512] f32
    preds_T [8, 14, 512, 512] f32
    gt_labels [8, 1, 512, 512] i32
and returns the scalar f32 loss. Batch dim is sharded across 8
NeuronCores (pure data parallel); each core emits per-column partial
sums of w*kl which the host reduces to the scalar.

Math (matches the reference up to fp reassociation):
  kl_pix = W/ZT + lnZS - lnZT, with
    ZT = sum_c exp(t_c), ZS = sum_c exp(s_c), W = sum_c exp(t_c)(t_c - s_c)
  loss = sum_p w_p * kl_p, where the per-pixel weight
    w_p = valid_{gt_p} / (C * max(n_{gt_p}, 1)) if p is a boundary pixel
  folds the per-class boundary masks, counts and validity — all derivable
  from gt_labels alone — into one plane computed on the host (gt is 3%
  of the input bytes; the preds math stays on device).

Numerics (validated host-side vs the f64 reference: rel err ~2e-5
against a 2e-2 gate):
  - Inputs are downconverted to f16 on the host (halves HBM traffic;
    inputs are N(0,1) so rounding noise washes out over ~240k boundary
    pixels).
  - exp(s) is only needed for the ZS channel-sum, so it tolerates a
    fast approximation: a Schraudolph-style bit-trick on the DVE
    (i16 = round(s*1024/ln2 + MU), bitcast to f16), with MU chosen so
    the relative error (+-2% sawtooth) is zero-mean over the uniform
    mantissa-phase distribution. This moves half the exp work off the
    ACT engine (the v2a bottleneck) onto a 4x-mode DVE tensor_scalar.
  - exp(t) stays exact on ACT (it also feeds the W stream).

On-device layout (per core, P = 262144 pixels):
  Channel-on-partition tiles [112, 2048]: partition = (pixgroup j in
  0..7, channel c in 0..13), each pixgroup row holds 2048 consecutive
  pixels. The 14-channel sums (ZT, ZS, W) are computed on the
  TensorEngine with a constant 0/1 selector lhsT so the PSUM output
  lands pixel-major: [128, 512] per 65536-pixel superchunk, psum row
  r', col f <-> pixel 65536*s + 512*r' + f.
  d = t - s runs on GPSIMD for 12 of 16 quads to balance DVE/Pool.
  Finals per superchunk: lnZT/lnZS/r=1/ZT on ACT, then on DVE
  g = lnZS - lnZT, h = W*r, kl = h + g, wkl = w*kl; one ones-column
  matmul accumulates per-column sums into a [1, 512] PSUM row; the
  host sums the 512 columns.
"""

import numpy as np
from contextlib import ExitStack

B, C, H, W = 8, 14, 512, 512
P = H * W              # 262144 pixels per sample
FQ = 2048              # free dim of a quad-tile
NSC = 4                # superchunks (65536 px each)
N_CORES = 8
WSCALE = 16384.0       # host weight scale: keeps w ~0.06 in f16

# Schraudolph f16 exp: i16 = round(x*SIG + MU); bitcast to f16.
# MU includes the zero-mean correction log2(E[(1+phi) 2^-phi]) so the
# +-2% piecewise-linear sawtooth has no systematic bias.
SIG = 1024.0 / float(np.log(2.0))
MU = 15301.0864

_CACHE = {}


def _build_sel() -> np.ndarray:
    """Phase-B selector weights [112, 16*128] f16.

    Partition layout is channel-major: p = ch*8 + j (so the input DMA
    iterates channels outermost and reads 8 contiguous pixgroup rows per
    channel). Block i' = 4*(Q%4) + c is the lhsT for matmul (quad Q,
    512-chunk c): sel[(ch*8+j), i', m] = 1 iff m == 32*(Q%4) + 4*j + c,
    mapping pixel 16384*Q + 2048*j + 512*c + f to psum row
    (32*(Q%4)+4*j+c), col f.
    """
    sel = np.zeros((112, 16, 128), np.float16)
    for qm in range(4):
        for c in range(4):
            blk = 4 * qm + c
            for j in range(8):
                row = 32 * qm + 4 * j + c
                sel[j::8, blk, row] = 1.0
    return np.ascontiguousarray(sel.reshape(112, 16 * 128))


def _patched_act_tables(orig_fn):
    """Force Exp and Ln to resolve to the one table set containing both
    (natural_log_exp_and_others) so the kernel never switches sets."""
    def wrapper(arch):
        import concourse.mybir as mybir
        tabs = orig_fn(arch)
        both = "natural_log_exp_and_others"
        if both in tabs:
            for name, funcs in tabs.items():
                if name != both:
                    funcs.discard(mybir.ActivationFunctionType.Exp)
                    funcs.discard(mybir.ActivationFunctionType.Ln)
        return tabs
    return wrapper


def _emit(nc, tc, S, T, WD, SEL, OUT):
    import concourse.bass as bass
    from concourse import mybir

    f32 = mybir.dt.float32
    f16 = mybir.dt.float16
    i16 = mybir.dt.int16
    Act = mybir.ActivationFunctionType
    Alu = mybir.AluOpType

    with ExitStack() as ctx:
        consts = ctx.enter_context(tc.tile_pool(name="consts", bufs=1))
        planes = ctx.enter_context(tc.tile_pool(name="planes", bufs=1))
        inpool = ctx.enter_context(tc.tile_pool(name="inpool", bufs=4))
        midpool = ctx.enter_context(tc.tile_pool(name="midpool", bufs=3))
        finpool = ctx.enter_context(tc.tile_pool(name="finpool", bufs=3))
        psum = ctx.enter_context(
            tc.tile_pool(name="psum", bufs=2, space=bass.MemorySpace.PSUM))
        psumc = ctx.enter_context(
            tc.tile_pool(name="psumc", bufs=1, space=bass.MemorySpace.PSUM))

        # ---- constants ----
        sel_sb = consts.tile([112, 16 * 128], f16)
        nc.sync.dma_start(sel_sb[:], SEL[:])
        ones_sb = consts.tile([128, 1], f16)
        nc.vector.memset(ones_sb[:], 1.0)
        # per-pixel weights, pixel-major per superchunk: [r, s, f]
        wt = consts.tile([128, NSC, 512], f16)
        nc.sync.dma_start(wt[:], WD[:].rearrange("r (s f) -> r s f", s=NSC))

        # ---- final reduction target: [1, 512] PSUM row ----
        acc = psumc.tile([1, 512], f32)

        Sr = S.rearrange("c (q j f) -> q c j f", j=8, f=FQ)
        Tr = T.rearrange("c (q j f) -> q c j f", j=8, f=FQ)

        def emit_quad(s, qq, psZT, psZS, psW):
            Q = 4 * s + qq
            St = inpool.tile([112, FQ], f16, tag="St")
            Tt = inpool.tile([112, FQ], f16, tag="Tt")
            nc.sync.dma_start(St[:], Sr[Q])
            nc.sync.dma_start(Tt[:], Tr[Q])
            d = midpool.tile([112, FQ], f16, tag="d")
            # 12 of 16 quads compute d on GPSIMD to unload the DVE
            deng = nc.vector if qq == 0 else nc.gpsimd
            deng.tensor_tensor(d[:], Tt[:], St[:], Alu.subtract)
            eT = midpool.tile([112, FQ], f16, tag="eT")
            nc.scalar.activation(eT[:], Tt[:], Act.Exp)
            eS = midpool.tile([112, FQ], i16, tag="eS")
            nc.vector.tensor_scalar(eS[:], St[:], SIG, MU, Alu.mult, Alu.add)
            eSf = eS[:].bitcast(f16)
            m = midpool.tile([112, FQ], f16, tag="m")
            nc.vector.tensor_mul(m[:], eT[:], d[:])
            for cc in range(4):
                blk = 4 * qq + cc
                selap = sel_sb[:, blk * 128:(blk + 1) * 128]
                st = (qq == 0 and cc == 0)
                sp = (qq == 3 and cc == 3)
                cs = slice(cc * 512, (cc + 1) * 512)
                nc.tensor.matmul(psZT[:], selap, eT[:, cs],
                                 start=st, stop=sp)
                nc.tensor.matmul(psZS[:], selap, eSf[:, cs],
                                 start=st, stop=sp)
                nc.tensor.matmul(psW[:], selap, m[:, cs],
                                 start=st, stop=sp)

        def make_finals_parts(s, psZT, psZS, psW):
            """Finals of superchunk s as closures, interleaved between the
            next superchunk's quads for a smoother static schedule."""
            st = {}

            def part0():
                lnZT = finpool.tile([128, 512], f32, tag="lnZT")
                lnZS = finpool.tile([128, 512], f32, tag="lnZS")
                r = finpool.tile([128, 512], f32, tag="r")
                nc.scalar.activation(lnZT[:], psZT[:], Act.Ln)
                nc.scalar.activation(lnZS[:], psZS[:], Act.Ln)
                nc.scalar.activation(r[:], lnZT[:], Act.Exp, scale=-1.0)
                st["lnZT"], st["lnZS"], st["r"] = lnZT, lnZS, r

            def part1():
                g = finpool.tile([128, 512], f16, tag="g")
                h = finpool.tile([128, 512], f16, tag="h")
                nc.vector.tensor_sub(g[:], st["lnZS"][:], st["lnZT"][:])
                nc.vector.tensor_mul(h[:], psW[:], st["r"][:])
                kl = finpool.tile([128, 512], f16, tag="kl")
                wkl = finpool.tile([128, 512], f16, tag="wkl")
                nc.vector.tensor_add(kl[:], h[:], g[:])
                nc.vector.tensor_mul(wkl[:], kl[:], wt[:, s, :])
                nc.tensor.matmul(acc[:], ones_sb[:], wkl[:],
                                 start=(s == 0), stop=(s == NSC - 1))

            return [part0, part1]

        # software pipeline: superchunk s's finals are emitted between
        # superchunk s+1's quads so no engine stalls on the
        # PE -> DVE -> ACT -> DVE -> PE round-trip at superchunk edges
        pending = None
        for s in range(NSC):
            psZT = psum.tile([128, 512], f32, tag="psZT")
            psZS = psum.tile([128, 512], f32, tag="psZS")
            psW = psum.tile([128, 512], f32, tag="psW")
            for qq in range(4):
                emit_quad(s, qq, psZT, psZS, psW)
                if pending is not None and qq < 2:
                    pending[qq]()
            pending = make_finals_parts(s, psZT, psZS, psW)
        for part in pending:
            part()

        acc_sb = planes.tile([1, 512], f32)
        nc.vector.tensor_copy(acc_sb[:], acc[:])
        nc.sync.dma_start(OUT[:], acc_sb[:])


def _build_nc():
    import concourse.bacc as bacc
    import concourse.tile as tile
    import concourse.hw_specs as hw_specs
    from concourse import mybir

    if not getattr(bacc, "_act_tables_patched", False):
        bacc.get_activation_tables = _patched_act_tables(
            hw_specs.get_activation_tables)
        bacc._act_tables_patched = True

    f32 = mybir.dt.float32
    f16 = mybir.dt.float16

    nc = bacc.Bacc("TRN2", target_bir_lowering=False, debug=False)
    S = nc.declare_dram_parameter("preds_s", [C, P], f16, isOutput=False)
    T = nc.declare_dram_parameter("preds_t", [C, P], f16, isOutput=False)
    WD = nc.declare_dram_parameter("wpix", [128, NSC * 512], f16,
                                   isOutput=False)
    SEL = nc.declare_dram_parameter("sel", [112, 16 * 128], f16,
                                    isOutput=False)
    OUT = nc.declare_dram_parameter("partials", [1, 512], f32, isOutput=True)
    with tile.TileContext(nc) as tc:
        _emit(nc, tc, S, T, WD, SEL, OUT)
    nc.compile()
    return nc


def _get_nc():
    if "nc" not in _CACHE:
        _CACHE["nc"] = _build_nc()
    return _CACHE["nc"]


def _weights(gt: np.ndarray) -> np.ndarray:
    """Per-pixel loss weights [128, NSC*512] f16 for one sample's gt.

    w_p = WSCALE * valid_k / (C * max(n_k, 1)) for boundary pixels of
    class k = gt_p (masks are disjoint across k), 0 elsewhere.
    Matches the reference: boundary = mask ^ erosion (cross, zero
    border); valid_k = (sum of flat boundary indices) > 0.
    Layout is pixel-major per superchunk: out[r, s*512+f] = w[128*s+r, f].
    """
    w = np.zeros((H, W), np.float32)
    mp = np.zeros((H + 2, W + 2), bool)
    for k in range(1, C):
        m = gt == k
        mp[1:-1, 1:-1] = m
        eroded = (m & mp[:-2, 1:-1] & mp[2:, 1:-1]
                  & mp[1:-1, :-2] & mp[1:-1, 2:])
        boundary = m ^ eroded
        n = int(boundary.sum())
        if n == 0:
            continue
        # valid iff any boundary pixel has flat index > 0
        if n == 1 and boundary[0, 0]:
            continue
        w[boundary] = WSCALE / (C * n)
    return np.ascontiguousarray(
        w.reshape(NSC, 128, 512).transpose(1, 0, 2).reshape(128, NSC * 512)
    ).astype(np.float16)


def make_in_maps(preds_S, preds_T, gt_labels):
    """Shard the full inputs into per-core input maps (host-side layout)."""
    gt = np.asarray(gt_labels)[:, 0]                       # [nb, 512, 512]
    nb = gt.shape[0]
    sel = _build_sel()
    pS = np.asarray(preds_S, np.float32).reshape(nb, C, P).astype(np.float16)
    pT = np.asarray(preds_T, np.float32).reshape(nb, C, P).astype(np.float16)
    return [
        {"preds_s": np.ascontiguousarray(pS[b]),
         "preds_t": np.ascontiguousarray(pT[b]),
         "wpix": _weights(gt[b]), "sel": sel}
        for b in range(nb)
    ]


def postprocess(partials_per_core) -> np.float32:
    """Reduce per-core [1, 512] partials to the scalar loss."""
    loss = 0.0
    for part in partials_per_core:
        loss += float(part.astype(np.float64).sum()) / WSCALE
    return np.float32(loss)


def _run(inputs, trace=False, trace_kwargs=None):
    from concourse.bass_utils import run_bass_kernel_spmd

    nc = _get_nc()
    in_maps = make_in_maps(inputs["preds_S"], inputs["preds_T"],
                           inputs["gt_labels"])
    res = run_bass_kernel_spmd(nc, in_maps, list(range(len(in_maps))),
                               trace=trace, **(trace_kwargs or {}))
    parts = [res.results[b]["partials"] for b in range(len(in_maps))]
    loss = postprocess(parts)
    return loss, res


def kernel(preds_S, preds_T, gt_labels):
    assert preds_S.shape == (B, C, H, W), preds_S.shape
    loss, _ = _run({"preds_S": preds_S, "preds_T": preds_T,
                    "gt_labels": gt_labels})
    return loss


# revision 14
# speedup vs baseline: 1.4593x; 1.2342x over previous
"""Trainium2 Bass kernel for nn_BoundaryKDV7 (boundary KL-divergence loss).

Contract: kernel(**inputs) takes the FULL inputs
    preds_S [8, 14, 512, 512] f32
    preds_T [8, 14, 512, 512] f32
    gt_labels [8, 1, 512, 512] i32
and returns the scalar f32 loss. Batch dim is sharded across 8
NeuronCores (pure data parallel); each core emits per-column partial
sums of w*kl which the host reduces to the scalar.

Math (matches the reference up to fp reassociation):
  kl_pix = W/ZT + lnZS - lnZT, with
    ZT = sum_c exp(t_c), ZS = sum_c exp(s_c), W = sum_c exp(t_c)(t_c - s_c)
  loss = sum_p w_p * kl_p, where the per-pixel weight
    w_p = valid_{gt_p} / (C * max(n_{gt_p}, 1)) if p is a boundary pixel
  folds the per-class boundary masks, counts and validity (derivable
  from gt_labels alone) into one plane computed on the host.

Numerics: f16 inputs (halves HBM traffic), exact exp(t) on ACT,
Schraudolph bit-trick exp(s) on DVE (i16 = round(s*1477.32 + 15301.09)
written through a bitcast view of an f16 tile; MU calibrated so the
+-2% sawtooth is zero-mean), f16 intermediates.

Trace-driven layout (v3):
  - No GPSIMD: its slow software tensor ops poison concurrent DVE ops
    4-7x via SBUF port contention.
  - Matmuls write 32-row PSUM slices with explicit tile_position
    (0, 32*qq): the PE's 32x32 sub-array col-tiling runs different
    quads' matmuls concurrently and LDWEIGHTS drops to 32 columns.
    Every quad's first chunk matmul carries start=True so its slice's
    has_written bits are cleared regardless of whether the hardware
    clear is bank-wide or per-col-tile (PSUM banks are reused across
    superchunk pairs; stale accumulation otherwise).
  - A zero-selector touch matmul per bank (accumulate 0 over the full
    [128, 512] AP) gives downstream ACT/DVE readers a whole-tile
    dependency on the 16 partition-sliced writes.
  - Elementwise ops process double-quad tiles [112, 2, 2048] (free
    4096) to amortize fixed per-instruction overhead.

On-device layout (per core, P = 262144 pixels):
  Channel-on-partition tiles: partition = (channel c)*8 + (pixgroup j),
  pixgroup j of quad Q holds pixels 16384*Q + 2048*j + [0,2048).
  TensorE 0/1-selector matmuls produce pixel-major channel sums:
  psum plane [128, 512] per 65536-pixel superchunk, row r', col f
  <-> pixel 65536*s + 512*r' + f; quad qq of the superchunk owns rows
  [32*qq, 32*qq+32), chunk cc owns rows == cc (mod 4).
  Finals per superchunk: lnZT/lnZS/r=1/ZT on ACT, then on DVE
  g = lnZS - lnZT, h = W*r, kl = h + g, wkl = w*kl; one ones-column
  matmul accumulates per-column sums into a [1, 512] PSUM row; the
  host sums the 512 columns.
"""

import numpy as np
from contextlib import ExitStack

B, C, H, W = 8, 14, 512, 512
P = H * W              # 262144 pixels per sample
FQ = 2048              # free dim of a quad-tile
NSC = 4                # superchunks (65536 px each)
N_CORES = 8
WSCALE = 16384.0       # host weight scale: keeps w ~0.06 in f16

# Schraudolph f16 exp: i16 = round(x*SIG + MU); bitcast to f16.
SIG = 1024.0 / float(np.log(2.0))
MU = 15301.0864

_CACHE = {}


def _build_sel() -> np.ndarray:
    """Selector weights [112, 4*32] f16. Block cc (chunk of 512 px) is
    the lhsT mapping partition p = c*8+j to slice-row m = 4*j + cc:
    quad qq's matmul for chunk cc writes psum rows 32*qq + 4*j + cc."""
    sel = np.zeros((112, 4, 32), np.float16)
    for cc in range(4):
        for j in range(8):
            sel[j::8, cc, 4 * j + cc] = 1.0
    return np.ascontiguousarray(sel.reshape(112, 4 * 32))


def _patched_act_tables(orig_fn):
    """Force Exp and Ln to resolve to the one table set containing both
    (natural_log_exp_and_others) so the kernel never switches sets."""
    def wrapper(arch):
        import concourse.mybir as mybir
        tabs = orig_fn(arch)
        both = "natural_log_exp_and_others"
        if both in tabs:
            for name, funcs in tabs.items():
                if name != both:
                    funcs.discard(mybir.ActivationFunctionType.Exp)
                    funcs.discard(mybir.ActivationFunctionType.Ln)
        return tabs
    return wrapper


def _emit(nc, tc, S, T, WD, SEL, OUT):
    import concourse.bass as bass
    from concourse import mybir

    f32 = mybir.dt.float32
    f16 = mybir.dt.float16
    i16 = mybir.dt.int16
    Act = mybir.ActivationFunctionType
    Alu = mybir.AluOpType

    with ExitStack() as ctx:
        consts = ctx.enter_context(tc.tile_pool(name="consts", bufs=1))
        planes = ctx.enter_context(tc.tile_pool(name="planes", bufs=1))
        inpool = ctx.enter_context(tc.tile_pool(name="inpool", bufs=3))
        midpool = ctx.enter_context(tc.tile_pool(name="midpool", bufs=2))
        finpool = ctx.enter_context(tc.tile_pool(name="finpool", bufs=3))
        psum = ctx.enter_context(
            tc.tile_pool(name="psum", bufs=2, space=bass.MemorySpace.PSUM))
        psumc = ctx.enter_context(
            tc.tile_pool(name="psumc", bufs=1, space=bass.MemorySpace.PSUM))

        # ---- constants ----
        sel_sb = consts.tile([112, 4 * 32], f16)
        nc.sync.dma_start(sel_sb[:], SEL[:])
        ones_sb = consts.tile([128, 1], f16)
        nc.vector.memset(ones_sb[:], 1.0)
        zsel_sb = consts.tile([112, 128], f16)
        nc.vector.memset(zsel_sb[:], 0.0)
        # per-pixel weights, pixel-major per superchunk: [r, s, f]
        wt = consts.tile([128, NSC, 512], f16)
        nc.sync.dma_start(wt[:], WD[:].rearrange("r (s f) -> r s f", s=NSC))

        # ---- final reduction target: [1, 512] PSUM row ----
        acc = psumc.tile([1, 512], f32)

        Sr = S.rearrange("c (q j f) -> q c j f", j=8, f=FQ)
        Tr = T.rearrange("c (q j f) -> q c j f", j=8, f=FQ)

        def emit_dq(dq, psZT, psZS, psW):
            St = inpool.tile([112, 2, FQ], f16, tag="St")
            Tt = inpool.tile([112, 2, FQ], f16, tag="Tt")
            for q2 in range(2):
                nc.sync.dma_start(St[:, q2, :], Sr[2 * dq + q2])
                nc.sync.dma_start(Tt[:, q2, :], Tr[2 * dq + q2])
            # elementwise stage on [112, 4096] views
            eT = midpool.tile([112, 2, FQ], f16, tag="eT")
            nc.scalar.activation(eT[:], Tt[:], Act.Exp)
            # Schraudolph exp(s): write the i16 fixed-point value THROUGH
            # a bitcast view of an f16 tile (write-via-bitcast is the
            # pattern Tile's dependency tracking understands, cf. memzero);
            # the PE then reads the tile natively as f16.
            eSf = midpool.tile([112, 2, FQ], f16, tag="eS")
            nc.vector.tensor_scalar(eSf[:].bitcast(i16), St[:], SIG, MU,
                                    Alu.mult, Alu.add)
            d = midpool.tile([112, 2, FQ], f16, tag="d")
            m = midpool.tile([112, 2, FQ], f16, tag="m")
            nc.vector.tensor_sub(d[:], Tt[:], St[:])
            nc.vector.tensor_mul(m[:], eT[:], d[:])
            # PE: per quad q2, 4 chunks x 3 streams into 32-row slices
            for q2 in range(2):
                qq = (2 * dq + q2) % 4     # quad index within superchunk
                rs = slice(32 * qq, 32 * qq + 32)
                tp = (0, 32 * qq)
                for cc in range(4):
                    selap = sel_sb[:, cc * 32:(cc + 1) * 32]
                    st = (cc == 0)
                    cs = slice(cc * 512, (cc + 1) * 512)
                    nc.tensor.matmul(psZT[rs, :], selap, eT[:, q2, cs],
                                     start=st, stop=False, tile_position=tp)
                    nc.tensor.matmul(psZS[rs, :], selap, eSf[:, q2, cs],
                                     start=st, stop=False, tile_position=tp)
                    nc.tensor.matmul(psW[rs, :], selap, m[:, q2, cs],
                                     start=st, stop=False, tile_position=tp)
                if qq == 3:
                    # zero-selector touch matmuls: accumulate 0 over the
                    # FULL [128, 512] psum AP so downstream readers (ACT
                    # Ln / DVE h) get a whole-tile dependency on all 16
                    # partition-sliced writes above.
                    zc = eT[:, 1, 0:512]
                    nc.tensor.matmul(psZT[:], zsel_sb[:], zc,
                                     start=False, stop=True)
                    nc.tensor.matmul(psZS[:], zsel_sb[:], zc,
                                     start=False, stop=True)
                    nc.tensor.matmul(psW[:], zsel_sb[:], zc,
                                     start=False, stop=True)

        def make_finals_parts(s, psZT, psZS, psW):
            """Finals of superchunk s as closures, interleaved between the
            next superchunk's double-quads for a smoother schedule."""
            st = {}

            def part0():
                lnZT = finpool.tile([128, 512], f32, tag="lnZT")
                lnZS = finpool.tile([128, 512], f32, tag="lnZS")
                r = finpool.tile([128, 512], f32, tag="r")
                nc.scalar.activation(lnZT[:], psZT[:], Act.Ln)
                nc.scalar.activation(lnZS[:], psZS[:], Act.Ln)
                nc.scalar.activation(r[:], lnZT[:], Act.Exp, scale=-1.0)
                st["lnZT"], st["lnZS"], st["r"] = lnZT, lnZS, r

            def part1():
                g = finpool.tile([128, 512], f16, tag="g")
                h = finpool.tile([128, 512], f16, tag="h")
                nc.vector.tensor_sub(g[:], st["lnZS"][:], st["lnZT"][:])
                nc.vector.tensor_mul(h[:], psW[:], st["r"][:])
                kl = finpool.tile([128, 512], f16, tag="kl")
                wkl = finpool.tile([128, 512], f16, tag="wkl")
                nc.vector.tensor_add(kl[:], h[:], g[:])
                nc.vector.tensor_mul(wkl[:], kl[:], wt[:, s, :])
                nc.tensor.matmul(acc[:], ones_sb[:], wkl[:],
                                 start=(s == 0), stop=(s == NSC - 1))

            return [part0, part1]

        # software pipeline: superchunk s's finals are emitted between
        # superchunk s+1's double-quads so no engine stalls on the
        # PE -> ACT -> DVE -> PE round-trip at superchunk edges
        pending = None
        for s in range(NSC):
            psZT = psum.tile([128, 512], f32, tag="psZT")
            psZS = psum.tile([128, 512], f32, tag="psZS")
            psW = psum.tile([128, 512], f32, tag="psW")
            for q2 in range(2):
                emit_dq(2 * s + q2, psZT, psZS, psW)
                if pending is not None:
                    pending[q2]()
            pending = make_finals_parts(s, psZT, psZS, psW)
        for part in pending:
            part()

        acc_sb = planes.tile([1, 512], f32)
        nc.vector.tensor_copy(acc_sb[:], acc[:])
        nc.sync.dma_start(OUT[:], acc_sb[:])


def _build_nc():
    import concourse.bacc as bacc
    import concourse.tile as tile
    import concourse.hw_specs as hw_specs
    from concourse import mybir

    if not getattr(bacc, "_act_tables_patched", False):
        bacc.get_activation_tables = _patched_act_tables(
            hw_specs.get_activation_tables)
        bacc._act_tables_patched = True

    f32 = mybir.dt.float32
    f16 = mybir.dt.float16

    nc = bacc.Bacc("TRN2", target_bir_lowering=False, debug=False)
    S = nc.declare_dram_parameter("preds_s", [C, P], f16, isOutput=False)
    T = nc.declare_dram_parameter("preds_t", [C, P], f16, isOutput=False)
    WD = nc.declare_dram_parameter("wpix", [128, NSC * 512], f16,
                                   isOutput=False)
    SEL = nc.declare_dram_parameter("sel", [112, 4 * 32], f16,
                                    isOutput=False)
    OUT = nc.declare_dram_parameter("partials", [1, 512], f32, isOutput=True)
    with tile.TileContext(nc) as tc:
        _emit(nc, tc, S, T, WD, SEL, OUT)
    nc.compile()
    return nc


def _get_nc():
    if "nc" not in _CACHE:
        _CACHE["nc"] = _build_nc()
    return _CACHE["nc"]


def _weights(gt: np.ndarray) -> np.ndarray:
    """Per-pixel loss weights [128, NSC*512] f16 for one sample's gt.

    w_p = WSCALE * valid_k / (C * max(n_k, 1)) for boundary pixels of
    class k = gt_p (masks are disjoint across k), 0 elsewhere.
    Matches the reference: boundary = mask ^ erosion (cross, zero
    border); valid_k = (sum of flat boundary indices) > 0.
    Layout is pixel-major per superchunk: out[r, s*512+f] = w[128*s+r, f].
    """
    w = np.zeros((H, W), np.float32)
    mp = np.zeros((H + 2, W + 2), bool)
    for k in range(1, C):
        m = gt == k
        mp[1:-1, 1:-1] = m
        eroded = (m & mp[:-2, 1:-1] & mp[2:, 1:-1]
                  & mp[1:-1, :-2] & mp[1:-1, 2:])
        boundary = m ^ eroded
        n = int(boundary.sum())
        if n == 0:
            continue
        # valid iff any boundary pixel has flat index > 0
        if n == 1 and boundary[0, 0]:
            continue
        w[boundary] = WSCALE / (C * n)
    return np.ascontiguousarray(
        w.reshape(NSC, 128, 512).transpose(1, 0, 2).reshape(128, NSC * 512)
    ).astype(np.float16)


def make_in_maps(preds_S, preds_T, gt_labels):
    """Shard the full inputs into per-core input maps (host-side layout)."""
    gt = np.asarray(gt_labels)[:, 0]                       # [nb, 512, 512]
    nb = gt.shape[0]
    sel = _build_sel()
    pS = np.asarray(preds_S, np.float32).reshape(nb, C, P).astype(np.float16)
    pT = np.asarray(preds_T, np.float32).reshape(nb, C, P).astype(np.float16)
    return [
        {"preds_s": np.ascontiguousarray(pS[b]),
         "preds_t": np.ascontiguousarray(pT[b]),
         "wpix": _weights(gt[b]), "sel": sel}
        for b in range(nb)
    ]


def postprocess(partials_per_core) -> np.float32:
    """Reduce per-core [1, 512] partials to the scalar loss."""
    loss = 0.0
    for part in partials_per_core:
        loss += float(part.astype(np.float64).sum()) / WSCALE
    return np.float32(loss)


def _run(inputs, trace=False, trace_kwargs=None):
    from concourse.bass_utils import run_bass_kernel_spmd

    nc = _get_nc()
    in_maps = make_in_maps(inputs["preds_S"], inputs["preds_T"],
                           inputs["gt_labels"])
    res = run_bass_kernel_spmd(nc, in_maps, list(range(len(in_maps))),
                               trace=trace, **(trace_kwargs or {}))
    parts = [res.results[b]["partials"] for b in range(len(in_maps))]
    loss = postprocess(parts)
    return loss, res


def kernel(preds_S, preds_T, gt_labels):
    assert preds_S.shape == (B, C, H, W), preds_S.shape
    loss, _ = _run({"preds_S": preds_S, "preds_T": preds_T,
                    "gt_labels": gt_labels})
    return loss


# revision 15
# speedup vs baseline: 1.4721x; 1.0088x over previous
"""Trainium2 Bass kernel for nn_BoundaryKDV7 (boundary KL-divergence loss).

Contract: kernel(**inputs) takes the FULL inputs
    preds_S [8, 14, 512, 512] f32
    preds_T [8, 14, 512, 512] f32
    gt_labels [8, 1, 512, 512] i32
and returns the scalar f32 loss. Batch dim is sharded across 8
NeuronCores (pure data parallel); each core emits per-column partial
sums of w*kl which the host reduces to the scalar.

Math (matches the reference up to fp reassociation):
  kl_pix = W/ZT + lnZS - lnZT, with
    ZT = sum_c exp(t_c), ZS = sum_c exp(s_c), W = sum_c exp(t_c)(t_c - s_c)
  loss = sum_p w_p * kl_p, where the per-pixel weight
    w_p = valid_{gt_p} / (C * max(n_{gt_p}, 1)) if p is a boundary pixel
  folds the per-class boundary masks, counts and validity (derivable
  from gt_labels alone) into one plane computed on the host.

Numerics: f16 inputs (halves HBM traffic), exact exp(t) on ACT,
Schraudolph bit-trick exp(s) on DVE (i16 = round(s*1477.32 + 15301.09)
written through a bitcast view of an f16 tile; MU calibrated so the
+-2% sawtooth is zero-mean), f16 intermediates.

Trace-driven layout (v3):
  - No GPSIMD: its slow software tensor ops poison concurrent DVE ops
    4-7x via SBUF port contention.
  - Matmuls write 32-row PSUM slices with explicit tile_position
    (0, 32*qq): the PE's 32x32 sub-array col-tiling runs different
    quads' matmuls concurrently and LDWEIGHTS drops to 32 columns.
    Every quad's first chunk matmul carries start=True so its slice's
    has_written bits are cleared regardless of whether the hardware
    clear is bank-wide or per-col-tile (PSUM banks are reused across
    superchunk pairs; stale accumulation otherwise).
  - A zero-selector touch matmul per bank (accumulate 0 over the full
    [128, 512] AP) gives downstream ACT/DVE readers a whole-tile
    dependency on the 16 partition-sliced writes.
  - Elementwise ops process double-quad tiles [112, 2, 2048] (free
    4096) to amortize fixed per-instruction overhead.

On-device layout (per core, P = 262144 pixels):
  Channel-on-partition tiles: partition = (channel c)*8 + (pixgroup j),
  pixgroup j of quad Q holds pixels 16384*Q + 2048*j + [0,2048).
  TensorE 0/1-selector matmuls produce pixel-major channel sums:
  psum plane [128, 512] per 65536-pixel superchunk, row r', col f
  <-> pixel 65536*s + 512*r' + f; quad qq of the superchunk owns rows
  [32*qq, 32*qq+32), chunk cc owns rows == cc (mod 4).
  Finals per superchunk: lnZT/lnZS/r=1/ZT on ACT, then on DVE
  g = lnZS - lnZT, h = W*r, kl = h + g, wkl = w*kl; one ones-column
  matmul accumulates per-column sums into a [1, 512] PSUM row; the
  host sums the 512 columns.
"""

import numpy as np
from contextlib import ExitStack

B, C, H, W = 8, 14, 512, 512
P = H * W              # 262144 pixels per sample
FQ = 2048              # free dim of a quad-tile
NSC = 4                # superchunks (65536 px each)
N_CORES = 8
WSCALE = 16384.0       # host weight scale: keeps w ~0.06 in f16

# Schraudolph f16 exp: i16 = round(x*SIG + MU); bitcast to f16.
SIG = 1024.0 / float(np.log(2.0))
MU = 15301.0864

_CACHE = {}


def _build_sel() -> np.ndarray:
    """Selector weights [112, 4*32] f16. Block cc (chunk of 512 px) is
    the lhsT mapping partition p = c*8+j to slice-row m = 4*j + cc:
    quad qq's matmul for chunk cc writes psum rows 32*qq + 4*j + cc."""
    sel = np.zeros((112, 4, 32), np.float16)
    for cc in range(4):
        for j in range(8):
            sel[j::8, cc, 4 * j + cc] = 1.0
    return np.ascontiguousarray(sel.reshape(112, 4 * 32))


def _patched_act_tables(orig_fn):
    """Force Exp and Ln to resolve to the one table set containing both
    (natural_log_exp_and_others) so the kernel never switches sets."""
    def wrapper(arch):
        import concourse.mybir as mybir
        tabs = orig_fn(arch)
        both = "natural_log_exp_and_others"
        if both in tabs:
            for name, funcs in tabs.items():
                if name != both:
                    funcs.discard(mybir.ActivationFunctionType.Exp)
                    funcs.discard(mybir.ActivationFunctionType.Ln)
        return tabs
    return wrapper


def _emit(nc, tc, S, T, WD, SEL, OUT):
    import concourse.bass as bass
    from concourse import mybir

    f32 = mybir.dt.float32
    f16 = mybir.dt.float16
    i16 = mybir.dt.int16
    Act = mybir.ActivationFunctionType
    Alu = mybir.AluOpType

    with ExitStack() as ctx:
        consts = ctx.enter_context(tc.tile_pool(name="consts", bufs=1))
        planes = ctx.enter_context(tc.tile_pool(name="planes", bufs=1))
        inpool = ctx.enter_context(tc.tile_pool(name="inpool", bufs=4))
        midpool = ctx.enter_context(tc.tile_pool(name="midpool", bufs=3))
        finpool = ctx.enter_context(tc.tile_pool(name="finpool", bufs=3))
        psum = ctx.enter_context(
            tc.tile_pool(name="psum", bufs=2, space=bass.MemorySpace.PSUM))
        psumc = ctx.enter_context(
            tc.tile_pool(name="psumc", bufs=1, space=bass.MemorySpace.PSUM))

        # ---- constants ----
        sel_sb = consts.tile([112, 4 * 32], f16)
        nc.sync.dma_start(sel_sb[:], SEL[:])
        ones_sb = consts.tile([128, 1], f16)
        nc.vector.memset(ones_sb[:], 1.0)
        zsel_sb = consts.tile([112, 128], f16)
        nc.vector.memset(zsel_sb[:], 0.0)
        # per-pixel weights, pixel-major per superchunk: [r, s, f]
        wt = consts.tile([128, NSC, 512], f16)
        nc.sync.dma_start(wt[:], WD[:].rearrange("r (s f) -> r s f", s=NSC))

        # ---- final reduction target: [1, 512] PSUM row ----
        acc = psumc.tile([1, 512], f32)

        Sr = S.rearrange("c (q j f) -> q c j f", j=8, f=FQ)
        Tr = T.rearrange("c (q j f) -> q c j f", j=8, f=FQ)

        def emit_dq(dq, psZT, psZS, psW):
            St = inpool.tile([112, 2, FQ], f16, tag="St")
            Tt = inpool.tile([112, 2, FQ], f16, tag="Tt")
            for q2 in range(2):
                nc.sync.dma_start(St[:, q2, :], Sr[2 * dq + q2])
                nc.sync.dma_start(Tt[:, q2, :], Tr[2 * dq + q2])
            # elementwise stage on [112, 4096] views
            eT = midpool.tile([112, 2, FQ], f16, tag="eT")
            nc.scalar.activation(eT[:], Tt[:], Act.Exp)
            # Schraudolph exp(s): write the i16 fixed-point value THROUGH
            # a bitcast view of an f16 tile (write-via-bitcast is the
            # pattern Tile's dependency tracking understands, cf. memzero);
            # the PE then reads the tile natively as f16.
            eSf = midpool.tile([112, 2, FQ], f16, tag="eS")
            nc.vector.tensor_scalar(eSf[:].bitcast(i16), St[:], SIG, MU,
                                    Alu.mult, Alu.add)
            d = midpool.tile([112, 2, FQ], f16, tag="d")
            m = midpool.tile([112, 2, FQ], f16, tag="m")
            nc.vector.tensor_sub(d[:], Tt[:], St[:])
            nc.vector.tensor_mul(m[:], eT[:], d[:])
            # PE: per quad q2, 4 chunks x 3 streams into 32-row slices
            for q2 in range(2):
                qq = (2 * dq + q2) % 4     # quad index within superchunk
                rs = slice(32 * qq, 32 * qq + 32)
                tp = (0, 32 * qq)
                for cc in range(4):
                    selap = sel_sb[:, cc * 32:(cc + 1) * 32]
                    st = (cc == 0)
                    cs = slice(cc * 512, (cc + 1) * 512)
                    nc.tensor.matmul(psZT[rs, :], selap, eT[:, q2, cs],
                                     start=st, stop=False, tile_position=tp)
                    nc.tensor.matmul(psZS[rs, :], selap, eSf[:, q2, cs],
                                     start=st, stop=False, tile_position=tp)
                    nc.tensor.matmul(psW[rs, :], selap, m[:, q2, cs],
                                     start=st, stop=False, tile_position=tp)
                if qq == 3:
                    # zero-selector touch matmuls: accumulate 0 over the
                    # FULL [128, 512] psum AP so downstream readers (ACT
                    # Ln / DVE h) get a whole-tile dependency on all 16
                    # partition-sliced writes above.
                    zc = eT[:, 1, 0:512]
                    nc.tensor.matmul(psZT[:], zsel_sb[:], zc,
                                     start=False, stop=True)
                    nc.tensor.matmul(psZS[:], zsel_sb[:], zc,
                                     start=False, stop=True)
                    nc.tensor.matmul(psW[:], zsel_sb[:], zc,
                                     start=False, stop=True)

        def make_finals_parts(s, psZT, psZS, psW):
            """Finals of superchunk s as closures, interleaved between the
            next superchunk's double-quads for a smoother schedule."""
            st = {}

            def part0():
                lnZT = finpool.tile([128, 512], f32, tag="lnZT")
                lnZS = finpool.tile([128, 512], f32, tag="lnZS")
                r = finpool.tile([128, 512], f32, tag="r")
                nc.scalar.activation(lnZT[:], psZT[:], Act.Ln)
                nc.scalar.activation(lnZS[:], psZS[:], Act.Ln)
                nc.scalar.activation(r[:], lnZT[:], Act.Exp, scale=-1.0)
                st["lnZT"], st["lnZS"], st["r"] = lnZT, lnZS, r

            def part1():
                g = finpool.tile([128, 512], f16, tag="g")
                h = finpool.tile([128, 512], f16, tag="h")
                nc.vector.tensor_sub(g[:], st["lnZS"][:], st["lnZT"][:])
                nc.vector.tensor_mul(h[:], psW[:], st["r"][:])
                kl = finpool.tile([128, 512], f16, tag="kl")
                wkl = finpool.tile([128, 512], f16, tag="wkl")
                nc.vector.tensor_add(kl[:], h[:], g[:])
                nc.vector.tensor_mul(wkl[:], kl[:], wt[:, s, :])
                nc.tensor.matmul(acc[:], ones_sb[:], wkl[:],
                                 start=(s == 0), stop=(s == NSC - 1))

            return [part0, part1]

        # software pipeline: superchunk s's finals are emitted between
        # superchunk s+1's double-quads so no engine stalls on the
        # PE -> ACT -> DVE -> PE round-trip at superchunk edges
        pending = None
        for s in range(NSC):
            psZT = psum.tile([128, 512], f32, tag="psZT")
            psZS = psum.tile([128, 512], f32, tag="psZS")
            psW = psum.tile([128, 512], f32, tag="psW")
            for q2 in range(2):
                emit_dq(2 * s + q2, psZT, psZS, psW)
                if pending is not None:
                    pending[q2]()
            pending = make_finals_parts(s, psZT, psZS, psW)
        for part in pending:
            part()

        acc_sb = planes.tile([1, 512], f32)
        nc.vector.tensor_copy(acc_sb[:], acc[:])
        nc.sync.dma_start(OUT[:], acc_sb[:])


def _build_nc():
    import concourse.bacc as bacc
    import concourse.tile as tile
    import concourse.hw_specs as hw_specs
    from concourse import mybir

    if not getattr(bacc, "_act_tables_patched", False):
        bacc.get_activation_tables = _patched_act_tables(
            hw_specs.get_activation_tables)
        bacc._act_tables_patched = True

    f32 = mybir.dt.float32
    f16 = mybir.dt.float16

    nc = bacc.Bacc("TRN2", target_bir_lowering=False, debug=False)
    S = nc.declare_dram_parameter("preds_s", [C, P], f16, isOutput=False)
    T = nc.declare_dram_parameter("preds_t", [C, P], f16, isOutput=False)
    WD = nc.declare_dram_parameter("wpix", [128, NSC * 512], f16,
                                   isOutput=False)
    SEL = nc.declare_dram_parameter("sel", [112, 4 * 32], f16,
                                    isOutput=False)
    OUT = nc.declare_dram_parameter("partials", [1, 512], f32, isOutput=True)
    with tile.TileContext(nc) as tc:
        _emit(nc, tc, S, T, WD, SEL, OUT)
    nc.compile()
    return nc


def _get_nc():
    if "nc" not in _CACHE:
        _CACHE["nc"] = _build_nc()
    return _CACHE["nc"]


def _weights(gt: np.ndarray) -> np.ndarray:
    """Per-pixel loss weights [128, NSC*512] f16 for one sample's gt.

    w_p = WSCALE * valid_k / (C * max(n_k, 1)) for boundary pixels of
    class k = gt_p (masks are disjoint across k), 0 elsewhere.
    Matches the reference: boundary = mask ^ erosion (cross, zero
    border); valid_k = (sum of flat boundary indices) > 0.
    Layout is pixel-major per superchunk: out[r, s*512+f] = w[128*s+r, f].
    """
    w = np.zeros((H, W), np.float32)
    mp = np.zeros((H + 2, W + 2), bool)
    for k in range(1, C):
        m = gt == k
        mp[1:-1, 1:-1] = m
        eroded = (m & mp[:-2, 1:-1] & mp[2:, 1:-1]
                  & mp[1:-1, :-2] & mp[1:-1, 2:])
        boundary = m ^ eroded
        n = int(boundary.sum())
        if n == 0:
            continue
        # valid iff any boundary pixel has flat index > 0
        if n == 1 and boundary[0, 0]:
            continue
        w[boundary] = WSCALE / (C * n)
    return np.ascontiguousarray(
        w.reshape(NSC, 128, 512).transpose(1, 0, 2).reshape(128, NSC * 512)
    ).astype(np.float16)


def make_in_maps(preds_S, preds_T, gt_labels):
    """Shard the full inputs into per-core input maps (host-side layout)."""
    gt = np.asarray(gt_labels)[:, 0]                       # [nb, 512, 512]
    nb = gt.shape[0]
    sel = _build_sel()
    pS = np.asarray(preds_S, np.float32).reshape(nb, C, P).astype(np.float16)
    pT = np.asarray(preds_T, np.float32).reshape(nb, C, P).astype(np.float16)
    return [
        {"preds_s": np.ascontiguousarray(pS[b]),
         "preds_t": np.ascontiguousarray(pT[b]),
         "wpix": _weights(gt[b]), "sel": sel}
        for b in range(nb)
    ]


def postprocess(partials_per_core) -> np.float32:
    """Reduce per-core [1, 512] partials to the scalar loss."""
    loss = 0.0
    for part in partials_per_core:
        loss += float(part.astype(np.float64).sum()) / WSCALE
    return np.float32(loss)


def _run(inputs, trace=False, trace_kwargs=None):
    from concourse.bass_utils import run_bass_kernel_spmd

    nc = _get_nc()
    in_maps = make_in_maps(inputs["preds_S"], inputs["preds_T"],
                           inputs["gt_labels"])
    res = run_bass_kernel_spmd(nc, in_maps, list(range(len(in_maps))),
                               trace=trace, **(trace_kwargs or {}))
    parts = [res.results[b]["partials"] for b in range(len(in_maps))]
    loss = postprocess(parts)
    return loss, res


def kernel(preds_S, preds_T, gt_labels):
    assert preds_S.shape == (B, C, H, W), preds_S.shape
    loss, _ = _run({"preds_S": preds_S, "preds_T": preds_T,
                    "gt_labels": gt_labels})
    return loss
